# revision 11
# baseline (speedup 1.0000x reference)
"""Multi-head attention (B=4, S=2048, D=768, 12 heads) on 8 TRN2 NeuronCores.

Sharding: data parallel over batch (4) x tensor parallel over heads (2 groups
of 6 heads) = 8 cores. Each core computes its (batch, head-group) slice:
  Q^T/K^T projections in [feat, seq] layout, V in [seq, feat] layout
  (with a memset ones column per head so the P@V matmul also produces the
  softmax denominator), transposed scores S^T[k,q] per head pair with
  row-tiled concurrent K=64 matmuls, exp on the scalar engine straight out
  of PSUM, P^T@... accumulation into O^T, and on-chip normalization.
PE-offloads keep the (bottleneck) tensor engine at pure matmul work: the
reciprocal-row broadcast runs on GPSIMD (partition_broadcast, f32), and the
V bias — which commutes with softmax averaging — is a per-partition DVE add
on the normalized output instead of a bias-row matmul.
Host side only reshapes/casts for sharding and un-transposes on gather.
"""

import numpy as np
import ml_dtypes

B, S, D = 4, 2048, 768
NH, HD = 12, 64
HPC = 6                 # heads per core
FPC = HPC * HD          # 384 features per core
VW = HPC * (HD + 1)     # 390: V width with per-head ones column
N_CORES = 8
BF16 = ml_dtypes.bfloat16

_PROGRAM = None


def _build_program():
    import concourse.bass as bass  # noqa: F401
    import concourse.mybir as mybir
    from concourse import bacc
    from concourse.tile import TileContext
    from contextlib import ExitStack

    F = mybir.dt.float32
    BF = mybir.dt.bfloat16
    EXP = mybir.ActivationFunctionType.Exp

    nc = bacc.Bacc("TRN2", target_bir_lowering=False, debug=False, num_devices=N_CORES)

    xt = nc.dram_tensor("xt", [D, S], BF, kind="ExternalInput")
    wqt = nc.dram_tensor("wqt", [D, FPC], BF, kind="ExternalInput")
    wkt = nc.dram_tensor("wkt", [D, FPC], BF, kind="ExternalInput")
    wvt = nc.dram_tensor("wvt", [D, VW], BF, kind="ExternalInput")
    bqk = nc.dram_tensor("bqk", [FPC, 3], F, kind="ExternalInput")
    out = nc.dram_tensor("out", [FPC, S], F, kind="ExternalOutput")

    KT = D // 128        # 6 contraction tiles for projections
    MT = FPC // 128      # 3 feature tiles (= head pairs)
    QC = S // 512        # 4 seq chunks of 512
    JT = S // 128        # 16 key tiles

    with TileContext(nc) as tc, ExitStack() as ctx:
        const = ctx.enter_context(tc.tile_pool(name="const", bufs=1))
        qkv = ctx.enter_context(tc.tile_pool(name="qkv", bufs=1))
        osb = ctx.enter_context(tc.tile_pool(name="osb", bufs=1))
        ppool = ctx.enter_context(tc.tile_pool(name="pt", bufs=4))
        small = ctx.enter_context(tc.tile_pool(name="small", bufs=4))
        ps_pr = ctx.enter_context(tc.tile_pool(name="pspr", bufs=2, space="PSUM"))
        ps_s = ctx.enter_context(tc.tile_pool(name="pss", bufs=2, space="PSUM"))
        ps_o = ctx.enter_context(tc.tile_pool(name="pso", bufs=1, space="PSUM"))

        # ---- stage inputs in SBUF with one consolidated DMA per tensor
        # (many small dma_starts serialize ~0.65us each on the sequencer);
        # xt arrives per seq-chunk, interleaved with the projections that
        # consume each chunk
        xt_all = const.tile([128, KT * S], BF, tag="xta", name="xta")
        xt_s = [xt_all[:, i * S:(i + 1) * S] for i in range(KT)]
        wq_all = const.tile([128, KT * FPC], BF, tag="wqa", name="wqa")
        wqt_s = [wq_all[:, i * FPC:(i + 1) * FPC] for i in range(KT)]
        wk_all = const.tile([128, KT * FPC], BF, tag="wka", name="wka")
        wkt_s = [wk_all[:, i * FPC:(i + 1) * FPC] for i in range(KT)]
        wv_all = const.tile([128, KT * VW], BF, tag="wva", name="wva")
        wvt_s = [wv_all[:, i * VW:(i + 1) * VW] for i in range(KT)]
        bqk_all = const.tile([128, MT * 3], F, tag="bqk", name="bqka")
        bq_s = [bqk_all[:, 3 * t_i:3 * t_i + 1] for t_i in range(MT)]
        bk_s = [bqk_all[:, 3 * t_i + 1:3 * t_i + 2] for t_i in range(MT)]
        bv_s = [bqk_all[:, 3 * t_i + 2:3 * t_i + 3] for t_i in range(MT)]

        # DMA order follows first use: Q/K weights and xt chunk 0 gate the
        # first projections; V weights and biases are needed slightly later
        nc.sync.dma_start(
            wq_all[:].rearrange("p (b c) -> p b c", b=KT),
            wqt[:].rearrange("(b p) c -> p b c", p=128))
        nc.sync.dma_start(
            wk_all[:].rearrange("p (b c) -> p b c", b=KT),
            wkt[:].rearrange("(b p) c -> p b c", p=128))
        nc.sync.dma_start(
            bqk_all[:].rearrange("p (t c) -> p t c", t=MT),
            bqk[:].rearrange("(t p) c -> p t c", p=128))

        # dummy exp so the ACT table set loads during the DMA prologue
        dummy = small.tile([1, 1], F, tag="dummy", name="dummy")
        nc.scalar.activation(dummy[:], bq_s[0][0:1, :], EXP)

        v_s = [qkv.tile([128, VW], BF, tag=f"v{m}", name=f"v{m}") for m in range(JT)]
        qt_s = [qkv.tile([128, S], BF, tag=f"q{t_i}", name=f"qt{t_i}") for t_i in range(MT)]
        kt_s = [qkv.tile([128, S], BF, tag=f"k{t_i}", name=f"kt{t_i}") for t_i in range(MT)]
        o_s = [osb.tile([128, S], F, tag=f"o{t_i}", name=f"ot{t_i}") for t_i in range(MT)]

        # ---- projection work, broken into single-matmul thunks so the PE
        # stream can interleave them into the attention pipeline
        def v_group_thunks(m, use_act=False):
            # V projection (natural [seq, feat+ones] layout). bv is applied
            # to the normalized output instead (it passes through softmax
            # averaging unchanged), so no bias-row matmul; the per-head ones
            # columns (softmax denominator) are memset after eviction.
            cell = {}

            def mk(kk):
                def thunk():
                    if "ps" not in cell:
                        cell["ps"] = ps_pr.tile([128, VW], F, tag="pr",
                                                name=f"psv{m}")
                    nc.tensor.matmul(
                        cell["ps"][:],
                        lhsT=xt_s[kk][:, m * 128:(m + 1) * 128],
                        rhs=wvt_s[kk][:], start=(kk == 0), stop=(kk == KT - 1))
                    if kk == KT - 1:
                        if use_act:
                            nc.scalar.copy(v_s[m][:], cell["ps"][:])
                        else:
                            nc.vector.tensor_copy(v_s[m][:], cell["ps"][:])
                        nc.vector.memset(
                            v_s[m][:].rearrange("p (h c) -> p h c", c=HD + 1)[
                                :, :, HD:HD + 1], 1.0)
                return thunk
            return [mk(kk) for kk in range(KT)]

        def qk_group_thunks(w_s, b_s, dst, p, qc, use_act=False):
            cell = {}

            def mk(kk):
                def thunk():
                    if "ps" not in cell:
                        cell["ps"] = ps_pr.tile([128, 512], F, tag="pr",
                                                name=f"psp{p}_{qc}")
                    nc.tensor.matmul(
                        cell["ps"][:],
                        lhsT=w_s[kk][:, p * 128:(p + 1) * 128],
                        rhs=xt_s[kk][:, qc * 512:(qc + 1) * 512],
                        start=(kk == 0), stop=(kk == KT - 1))
                    if kk == KT - 1:
                        if use_act:
                            nc.scalar.add(
                                dst[p][:, qc * 512:(qc + 1) * 512],
                                cell["ps"][:], b_s[p][:])
                        else:
                            nc.vector.tensor_scalar_add(
                                dst[p][:, qc * 512:(qc + 1) * 512],
                                cell["ps"][:], b_s[p][:])
                return thunk
            return [mk(kk) for kk in range(KT)]

        def qk_pair_thunks(p):
            th = []
            for w_s, b_s, dst in ((wqt_s, bq_s, qt_s), (wkt_s, bk_s, kt_s)):
                for qc in range(QC):
                    th.extend(qk_group_thunks(w_s, b_s, dst, p, qc))
            return th

        # xt chunk DMAs issued up front (DMA engines are otherwise idle);
        # chunk 0 lands first so the pair-0 chunk-0 projections can start
        # xt goes via SWDGE (gpsimd) so it runs in parallel with the weight
        # DMAs on the HWDGE ring
        for qc in range(QC):
            nc.gpsimd.dma_start(
                xt_all[:].rearrange("p (b c) -> p b c", b=KT)[
                    :, :, qc * 512:(qc + 1) * 512],
                xt[:].rearrange("(b p) c -> p b c", p=128)[
                    :, :, qc * 512:(qc + 1) * 512])
        nc.sync.dma_start(
            wv_all[:].rearrange("p (b c) -> p b c", b=KT),
            wvt[:].rearrange("(b p) c -> p b c", p=128))

        # prologue compute: only what gates the very first attention step --
        # the pair-0 chunk-0 Q/K projections. Everything else drips into the
        # PE stream during the attention pipeline, ordered by when it is
        # first consumed (V tiles by k-step, K chunks early, Q chunks by
        # q-chunk, later pairs last).
        for w_s, b_s, dst in ((wqt_s, bq_s, qt_s), (wkt_s, bk_s, kt_s)):
            for th in qk_group_thunks(w_s, b_s, dst, 0, qc=0):
                th()

        # Tile tracks dependencies in EMISSION order, so every projection
        # thunk must be emitted strictly before its first consumer. Each
        # thunk gets a deadline (step index); the drip drains all due thunks
        # plus up to 3 more per step to smooth PE load.
        from collections import deque
        items = []   # (deadline, order, thunk)

        def add(deadline, thunks):
            for th in thunks:
                items.append((deadline, len(items), th))

        for m in range(JT):
            add(m, v_group_thunks(m))                       # PV(0,0,m) at step m
        for qc in range(1, QC):
            # kt chunk qc feeds scores(0,*,4qc..) first emitted at step 4qc-1
            add(max(0, 4 * qc - 2),
                qk_group_thunks(wkt_s, bk_s, kt_s, 0, qc))
            # qt chunk qc feeds scores(0,qc,0) first emitted at step 16qc-1
            add(max(0, 16 * qc - 2),
                qk_group_thunks(wqt_s, bq_s, qt_s, 0, qc))
        for p in range(1, MT):
            base = 64 * p
            for qc in range(QC):
                add(base + 4 * qc - 2,
                    qk_group_thunks(wkt_s, bk_s, kt_s, p, qc))
                add(base + 16 * qc - 2,
                    qk_group_thunks(wqt_s, bq_s, qt_s, p, qc))
        items.sort(key=lambda x: (x[0], x[1]))
        proj_q = deque(items)

        # ---- attention pipeline over flattened (pair, q-chunk, k-tile) steps
        steps = [(p, qc, j) for p in range(MT) for qc in range(QC)
                 for j in range(JT)]

        def emit_scores(p, qc, j):
            sp = ps_s.tile([128, 1024], F, tag="s", name=f"s{p}_{qc}_{j}")
            for h in range(2):
                nc.tensor.matmul(
                    sp[:, h * 512:(h + 1) * 512],
                    lhsT=kt_s[p][h * 64:(h + 1) * 64, j * 128:(j + 1) * 128],
                    rhs=qt_s[p][h * 64:(h + 1) * 64, qc * 512:(qc + 1) * 512],
                    start=True, stop=True, tile_position=(h * 64, 0))
            return sp

        def make_norm(p, qc, osb, recs, s, final=False):
            # Broadcast of the reciprocal row runs on the (otherwise idle)
            # GPSIMD engine instead of a PE matmul + DVE copy; normalize
            # multiply + output store deferred into the next step so it
            # never delays the scores stream.
            def norm():
                for h in range(2):
                    bc_sb = small.tile([64, 512], F, tag="bc", name=f"bcs{s}_{h}")
                    nc.gpsimd.partition_broadcast(bc_sb[:], recs[h][:])
                    nc.vector.tensor_mul(
                        o_s[p][h * 64:(h + 1) * 64, qc * 512:(qc + 1) * 512],
                        osb[h][:], bc_sb[:])
                nc.vector.tensor_scalar_add(
                    o_s[p][:, qc * 512:(qc + 1) * 512],
                    o_s[p][:, qc * 512:(qc + 1) * 512], bv_s[p])
                nc.sync.dma_start(
                    out[p * 128:(p + 1) * 128, qc * 512:(qc + 1) * 512],
                    o_s[p][:, qc * 512:(qc + 1) * 512])
            return norm

        sp_next = emit_scores(*steps[0])
        Os = None
        pending_norm = None
        for s, (p, qc, j) in enumerate(steps):
            sp = sp_next
            if s + 1 < len(steps):
                sp_next = emit_scores(*steps[s + 1])
            if pending_norm is not None:
                pending_norm()
                pending_norm = None
            # drip projection work into the PE stream: everything due by this
            # step (correctness), plus up to 2 thunks to smooth PE load
            extra = 2
            while proj_q and (proj_q[0][0] <= s or extra > 0):
                if proj_q[0][0] > s:
                    extra -= 1
                proj_q.popleft()[2]()
            if j == 0:
                O0 = ps_o.tile([65, 512], F, tag="o0", name=f"o0_{p}_{qc}")
                O1 = ps_o.tile([65, 512], F, tag="o1", name=f"o1_{p}_{qc}")
                Os = (O0, O1)
            pt = ppool.tile([128, 1024], BF, tag="p", name=f"pt{s}")
            nc.scalar.activation(pt[:], sp[:], EXP, scale=0.125)
            for h in range(2):
                lh = 2 * p + h
                nc.tensor.matmul(
                    Os[h][:],
                    lhsT=v_s[j][:, lh * 65:(lh + 1) * 65],
                    rhs=pt[:, h * 512:(h + 1) * 512],
                    start=(j == 0), stop=(j == JT - 1))
            if j == JT - 1:
                # evacuate the O banks promptly: reciprocal of the sum(exp)
                # row and a copy of the O rows to SBUF (both DVE); the
                # broadcast + multiply are deferred to the next step
                final = s == len(steps) - 1
                recs, osb = [], []
                for h in range(2):
                    rec = small.tile([1, 512], F, tag="rec", name=f"rec{s}_{h}")
                    nc.vector.reciprocal(rec[:], Os[h][64:65, :])
                    recs.append(rec)
                    ocp = small.tile([64, 512], F, tag=f"oc{h}", name=f"oc{s}_{h}")
                    if final:
                        nc.scalar.copy(ocp[:], Os[h][0:64, :])
                    else:
                        nc.vector.tensor_copy(ocp[:], Os[h][0:64, :])
                    osb.append(ocp)
                pending_norm = make_norm(p, qc, osb, recs, s, final=final)
        pending_norm()

    nc.compile()
    return nc


def _get_program():
    global _PROGRAM
    if _PROGRAM is None:
        _PROGRAM = _build_program()
    return _PROGRAM


def _prep_core_inputs(inputs, Wq, bq, Wk, bk, Wv, bv, core):
    b, g = divmod(core, 2)
    hs = slice(g * FPC, (g + 1) * FPC)
    xt = np.ascontiguousarray(inputs[b].T).astype(BF16)
    wqt = np.ascontiguousarray(Wq[hs, :].T).astype(BF16)
    wkt = np.ascontiguousarray(Wk[hs, :].T).astype(BF16)
    wvt = np.zeros((D, VW), dtype=BF16)
    for l in range(HPC):
        gh = g * HPC + l
        wvt[:, l * 65:l * 65 + 64] = Wv[gh * 64:(gh + 1) * 64, :].T.astype(BF16)
    bqk = np.stack([np.asarray(bq[hs], dtype=np.float32),
                    np.asarray(bk[hs], dtype=np.float32),
                    np.asarray(bv[hs], dtype=np.float32)], axis=1)
    return {
        "xt": xt,
        "wqt": wqt,
        "wkt": wkt,
        "wvt": wvt,
        "bqk": np.ascontiguousarray(bqk),
    }


def kernel(inputs, Wq, bq, Wk, bk, Wv, bv, _trace=False):
    from concourse.bass_utils import run_bass_kernel_spmd

    inputs = np.asarray(inputs, dtype=np.float32)
    Wq, Wk, Wv = (np.asarray(w, dtype=np.float32) for w in (Wq, Wk, Wv))
    bq, bk, bv = (np.asarray(b, dtype=np.float32) for b in (bq, bk, bv))
    in_maps = [
        _prep_core_inputs(inputs, Wq, bq, Wk, bk, Wv, bv, c) for c in range(N_CORES)
    ]
    nc = _get_program()
    res = run_bass_kernel_spmd(nc, in_maps, list(range(N_CORES)), trace=_trace)
    full = np.empty((B, S, D), dtype=np.float32)
    for c in range(N_CORES):
        b, g = divmod(c, 2)
        full[b, :, g * FPC:(g + 1) * FPC] = res.results[c]["out"].T
    if _trace:
        return full, res
    return full



# revision 16
# speedup vs baseline: 14.6378x; 14.6378x over previous
"""Multi-head attention (B=4, S=2048, D=768, 12 heads) on 8 TRN2 NeuronCores.

Sharding: data parallel over batch (4) x tensor parallel over heads (2 groups
of 6 heads) = 8 cores. Each core computes its (batch, head-group) slice:
  Q^T/K^T projections in [feat, seq] layout, V in [seq, feat] layout
  (with a memset ones column per head so the P@V matmul also produces the
  softmax denominator), transposed scores S^T[k,q] per head pair with
  row-tiled concurrent K=64 matmuls, exp on the scalar engine straight out
  of PSUM, P^T@... accumulation into O^T, and on-chip normalization.
PE-offloads keep the (bottleneck) tensor engine at pure matmul work: the
reciprocal-row broadcast runs on GPSIMD (partition_broadcast, f32), and the
V bias — which commutes with softmax averaging — is a per-partition DVE add
on the normalized output instead of a bias-row matmul.
Host side only reshapes/casts for sharding and un-transposes on gather.
"""

import numpy as np
import ml_dtypes

B, S, D = 4, 2048, 768
NH, HD = 12, 64
HPC = 6                 # heads per core
FPC = HPC * HD          # 384 features per core
VW = HPC * (HD + 1)     # 390: V width with per-head ones column
N_CORES = 8
BF16 = ml_dtypes.bfloat16

_PROGRAM = None


def _build_program():
    import concourse.bass as bass  # noqa: F401
    import concourse.mybir as mybir
    from concourse import bacc
    from concourse.tile import TileContext
    from contextlib import ExitStack

    F = mybir.dt.float32
    BF = mybir.dt.bfloat16
    EXP = mybir.ActivationFunctionType.Exp

    nc = bacc.Bacc("TRN2", target_bir_lowering=False, debug=False, num_devices=N_CORES)

    xt = nc.dram_tensor("xt", [D, S], BF, kind="ExternalInput")
    wqt = nc.dram_tensor("wqt", [D, FPC], BF, kind="ExternalInput")
    wkt = nc.dram_tensor("wkt", [D, FPC], BF, kind="ExternalInput")
    wvt = nc.dram_tensor("wvt", [D, VW], BF, kind="ExternalInput")
    bqk = nc.dram_tensor("bqk", [FPC, 3], F, kind="ExternalInput")
    out = nc.dram_tensor("out", [FPC, S], F, kind="ExternalOutput")

    KT = D // 128        # 6 contraction tiles for projections
    MT = FPC // 128      # 3 feature tiles (= head pairs)
    QC = S // 512        # 4 seq chunks of 512
    JT = S // 128        # 16 key tiles

    with TileContext(nc) as tc, ExitStack() as ctx:
        const = ctx.enter_context(tc.tile_pool(name="const", bufs=1))
        qkv = ctx.enter_context(tc.tile_pool(name="qkv", bufs=1))
        osb = ctx.enter_context(tc.tile_pool(name="osb", bufs=1))
        ppool = ctx.enter_context(tc.tile_pool(name="pt", bufs=4))
        small = ctx.enter_context(tc.tile_pool(name="small", bufs=4))
        ps_pr = ctx.enter_context(tc.tile_pool(name="pspr", bufs=2, space="PSUM"))
        ps_s = ctx.enter_context(tc.tile_pool(name="pss", bufs=2, space="PSUM"))
        ps_o = ctx.enter_context(tc.tile_pool(name="pso", bufs=1, space="PSUM"))

        # ---- stage inputs in SBUF with one consolidated DMA per tensor
        # (many small dma_starts serialize ~0.65us each on the sequencer);
        # xt arrives per seq-chunk, interleaved with the projections that
        # consume each chunk
        xt_all = const.tile([128, KT * S], BF, tag="xta", name="xta")
        xt_s = [xt_all[:, i * S:(i + 1) * S] for i in range(KT)]
        wq_all = const.tile([128, KT * FPC], BF, tag="wqa", name="wqa")
        wqt_s = [wq_all[:, i * FPC:(i + 1) * FPC] for i in range(KT)]
        wk_all = const.tile([128, KT * FPC], BF, tag="wka", name="wka")
        wkt_s = [wk_all[:, i * FPC:(i + 1) * FPC] for i in range(KT)]
        wv_all = const.tile([128, KT * VW], BF, tag="wva", name="wva")
        wvt_s = [wv_all[:, i * VW:(i + 1) * VW] for i in range(KT)]
        bqk_all = const.tile([128, MT * 3], F, tag="bqk", name="bqka")
        bq_s = [bqk_all[:, 3 * t_i:3 * t_i + 1] for t_i in range(MT)]
        bk_s = [bqk_all[:, 3 * t_i + 1:3 * t_i + 2] for t_i in range(MT)]
        bv_s = [bqk_all[:, 3 * t_i + 2:3 * t_i + 3] for t_i in range(MT)]

        # First-use-sliced DMA order: the very first projection group needs
        # only wq ktile 0 and xt (chunk 0, ktile 0) — land those (and the
        # tiny bias tile, which unblocks the ACT table preload) before the
        # bulk weight streams so the PE and ACT prologues start ~3.5us
        # earlier.
        nc.sync.dma_start(wq_all[:, 0:FPC], wqt[0:128, :])
        nc.gpsimd.dma_start(xt_all[:, 0:512], xt[0:128, 0:512])
        nc.sync.dma_start(
            bqk_all[:].rearrange("p (t c) -> p t c", t=MT),
            bqk[:].rearrange("(t p) c -> p t c", p=128))
        nc.sync.dma_start(
            wq_all[:, FPC:].rearrange("p (b c) -> p b c", b=KT - 1),
            wqt[128:, :].rearrange("(b p) c -> p b c", p=128))
        nc.sync.dma_start(
            wk_all[:].rearrange("p (b c) -> p b c", b=KT),
            wkt[:].rearrange("(b p) c -> p b c", p=128))

        # dummy exp so the ACT table set loads during the DMA prologue
        dummy = small.tile([1, 1], F, tag="dummy", name="dummy")
        nc.scalar.activation(dummy[:], bq_s[0][0:1, :], EXP)

        v_s = [qkv.tile([128, VW], BF, tag=f"v{m}", name=f"v{m}") for m in range(JT)]
        qt_s = [qkv.tile([128, S], BF, tag=f"q{t_i}", name=f"qt{t_i}") for t_i in range(MT)]
        kt_s = [qkv.tile([128, S], BF, tag=f"k{t_i}", name=f"kt{t_i}") for t_i in range(MT)]
        o_s = [osb.tile([128, S], F, tag=f"o{t_i}", name=f"ot{t_i}") for t_i in range(MT)]

        # ---- projection work, broken into single-matmul thunks so the PE
        # stream can interleave them into the attention pipeline
        def v_group_thunks(m, use_act=False):
            # V projection (natural [seq, feat+ones] layout). bv is applied
            # to the normalized output instead (it passes through softmax
            # averaging unchanged), so no bias-row matmul; the per-head ones
            # columns (softmax denominator) are memset after eviction.
            cell = {}

            def mk(kk):
                def thunk():
                    if "ps" not in cell:
                        cell["ps"] = ps_pr.tile([128, VW], F, tag="pr",
                                                name=f"psv{m}")
                    nc.tensor.matmul(
                        cell["ps"][:],
                        lhsT=xt_s[kk][:, m * 128:(m + 1) * 128],
                        rhs=wvt_s[kk][:], start=(kk == 0), stop=(kk == KT - 1))
                    if kk == KT - 1:
                        if use_act:
                            nc.scalar.copy(v_s[m][:], cell["ps"][:])
                        else:
                            nc.vector.tensor_copy(v_s[m][:], cell["ps"][:])
                        nc.vector.memset(
                            v_s[m][:].rearrange("p (h c) -> p h c", c=HD + 1)[
                                :, :, HD:HD + 1], 1.0)
                return thunk
            return [mk(kk) for kk in range(KT)]

        def qk_group_thunks(w_s, b_s, dst, p, qc, use_act=False):
            cell = {}

            def mk(kk):
                def thunk():
                    if "ps" not in cell:
                        cell["ps"] = ps_pr.tile([128, 512], F, tag="pr",
                                                name=f"psp{p}_{qc}")
                    nc.tensor.matmul(
                        cell["ps"][:],
                        lhsT=w_s[kk][:, p * 128:(p + 1) * 128],
                        rhs=xt_s[kk][:, qc * 512:(qc + 1) * 512],
                        start=(kk == 0), stop=(kk == KT - 1))
                    if kk == KT - 1:
                        if use_act:
                            nc.scalar.add(
                                dst[p][:, qc * 512:(qc + 1) * 512],
                                cell["ps"][:], b_s[p][:])
                        else:
                            nc.vector.tensor_scalar_add(
                                dst[p][:, qc * 512:(qc + 1) * 512],
                                cell["ps"][:], b_s[p][:])
                return thunk
            return [mk(kk) for kk in range(KT)]

        def qk_pair_thunks(p):
            th = []
            for w_s, b_s, dst in ((wqt_s, bq_s, qt_s), (wkt_s, bk_s, kt_s)):
                for qc in range(QC):
                    th.extend(qk_group_thunks(w_s, b_s, dst, p, qc))
            return th

        # xt chunk DMAs issued up front (DMA engines are otherwise idle);
        # chunk 0's remaining ktiles land first (ktile 0 was in the
        # prologue) so the pair-0 chunk-0 projections can run through.
        # xt goes via SWDGE (gpsimd) so it runs in parallel with the weight
        # DMAs on the HWDGE ring
        nc.gpsimd.dma_start(
            xt_all[:].rearrange("p (b c) -> p b c", b=KT)[:, 1:, 0:512],
            xt[:].rearrange("(b p) c -> p b c", p=128)[:, 1:, 0:512])
        for qc in range(1, QC):
            nc.gpsimd.dma_start(
                xt_all[:].rearrange("p (b c) -> p b c", b=KT)[
                    :, :, qc * 512:(qc + 1) * 512],
                xt[:].rearrange("(b p) c -> p b c", p=128)[
                    :, :, qc * 512:(qc + 1) * 512])
        nc.sync.dma_start(
            wv_all[:].rearrange("p (b c) -> p b c", b=KT),
            wvt[:].rearrange("(b p) c -> p b c", p=128))

        # prologue compute: only what gates the very first attention step --
        # the pair-0 chunk-0 Q/K projections. Everything else drips into the
        # PE stream during the attention pipeline, ordered by when it is
        # first consumed (V tiles by k-step, K chunks early, Q chunks by
        # q-chunk, later pairs last).
        for w_s, b_s, dst in ((wqt_s, bq_s, qt_s), (wkt_s, bk_s, kt_s)):
            for th in qk_group_thunks(w_s, b_s, dst, 0, qc=0):
                th()

        # Tile tracks dependencies in EMISSION order, so every projection
        # thunk must be emitted strictly before its first consumer. Each
        # thunk gets a deadline (step index); the drip drains all due thunks
        # plus up to 3 more per step to smooth PE load.
        from collections import deque
        items = []   # (deadline, order, thunk)

        def add(deadline, thunks):
            for th in thunks:
                items.append((deadline, len(items), th))

        for m in range(JT):
            add(m, v_group_thunks(m))                       # PV(0,0,m) at step m
        for qc in range(1, QC):
            # kt chunk qc feeds scores(0,*,4qc..) first emitted at step 4qc-1
            add(max(0, 4 * qc - 2),
                qk_group_thunks(wkt_s, bk_s, kt_s, 0, qc))
            # qt chunk qc feeds scores(0,qc,0) first emitted at step 16qc-1
            add(max(0, 16 * qc - 2),
                qk_group_thunks(wqt_s, bq_s, qt_s, 0, qc))
        for p in range(1, MT):
            base = 64 * p
            for qc in range(QC):
                add(base + 4 * qc - 2,
                    qk_group_thunks(wkt_s, bk_s, kt_s, p, qc))
                add(base + 16 * qc - 2,
                    qk_group_thunks(wqt_s, bq_s, qt_s, p, qc))
        items.sort(key=lambda x: (x[0], x[1]))
        proj_q = deque(items)

        # ---- attention pipeline over flattened (pair, q-chunk, k-tile) steps
        steps = [(p, qc, j) for p in range(MT) for qc in range(QC)
                 for j in range(JT)]

        def emit_scores(p, qc, j):
            sp = ps_s.tile([128, 1024], F, tag="s", name=f"s{p}_{qc}_{j}")
            for h in range(2):
                nc.tensor.matmul(
                    sp[:, h * 512:(h + 1) * 512],
                    lhsT=kt_s[p][h * 64:(h + 1) * 64, j * 128:(j + 1) * 128],
                    rhs=qt_s[p][h * 64:(h + 1) * 64, qc * 512:(qc + 1) * 512],
                    start=True, stop=True, tile_position=(h * 64, 0))
            return sp

        def make_norm(p, qc, osb, recs, s, final=False):
            # Broadcast of the reciprocal row runs on the (otherwise idle)
            # GPSIMD engine instead of a PE matmul + DVE copy; normalize
            # multiply + output store deferred into the next step so it
            # never delays the scores stream.
            def norm():
                for h in range(2):
                    bc_sb = small.tile([64, 512], F, tag="bc", name=f"bcs{s}_{h}")
                    nc.gpsimd.partition_broadcast(bc_sb[:], recs[h][:])
                    nc.vector.tensor_mul(
                        o_s[p][h * 64:(h + 1) * 64, qc * 512:(qc + 1) * 512],
                        osb[h][0:64, :], bc_sb[:])
                nc.vector.tensor_scalar_add(
                    o_s[p][:, qc * 512:(qc + 1) * 512],
                    o_s[p][:, qc * 512:(qc + 1) * 512], bv_s[p])
                nc.sync.dma_start(
                    out[p * 128:(p + 1) * 128, qc * 512:(qc + 1) * 512],
                    o_s[p][:, qc * 512:(qc + 1) * 512])
            return norm

        sp_next = emit_scores(*steps[0])
        Os = None
        pending_norm = None
        for s, (p, qc, j) in enumerate(steps):
            sp = sp_next
            if s + 1 < len(steps):
                sp_next = emit_scores(*steps[s + 1])
            if pending_norm is not None:
                pending_norm()
                pending_norm = None
            # drip projection work into the PE stream: everything due by this
            # step (correctness), plus up to 2 thunks to smooth PE load
            extra = 2
            while proj_q and (proj_q[0][0] <= s or extra > 0):
                if proj_q[0][0] > s:
                    extra -= 1
                proj_q.popleft()[2]()
            if j == 0:
                O0 = ps_o.tile([65, 512], F, tag="o0", name=f"o0_{p}_{qc}")
                O1 = ps_o.tile([65, 512], F, tag="o1", name=f"o1_{p}_{qc}")
                Os = (O0, O1)
            pt = ppool.tile([128, 1024], BF, tag="p", name=f"pt{s}")
            nc.scalar.activation(pt[:], sp[:], EXP, scale=0.125)
            for h in range(2):
                lh = 2 * p + h
                nc.tensor.matmul(
                    Os[h][:],
                    lhsT=v_s[j][:, lh * 65:(lh + 1) * 65],
                    rhs=pt[:, h * 512:(h + 1) * 512],
                    start=(j == 0), stop=(j == JT - 1))
            if j == JT - 1:
                # evacuate the O banks promptly: copy ALL 65 rows (output +
                # sum(exp) denominator row) to SBUF first so the PSUM banks
                # free as early as possible for the next group's PV start;
                # the reciprocals then read the denominator from SBUF, off
                # the bank-release critical path. Broadcast + multiply are
                # deferred to the next step.
                final = s == len(steps) - 1
                recs, osb = [], []
                if final:
                    # tail: only chain latency matters — reciprocals (DVE)
                    # read the denominator rows straight from PSUM while the
                    # ACT (idle in the tail) copies the O rows in parallel
                    for h in range(2):
                        rec = small.tile([1, 512], F, tag="rec",
                                         name=f"rec{s}_{h}")
                        nc.vector.reciprocal(rec[:], Os[h][64:65, :])
                        recs.append(rec)
                        ocp = small.tile([65, 512], F, tag=f"oc{h}",
                                         name=f"oc{s}_{h}")
                        nc.scalar.copy(ocp[0:64, :], Os[h][0:64, :])
                        osb.append(ocp)
                else:
                    # steady state: copy ALL 65 rows (output + denominator)
                    # to SBUF first so the PSUM banks free as early as
                    # possible for the next group's PV start; reciprocals
                    # then read from SBUF, off the bank-release path
                    for h in range(2):
                        ocp = small.tile([65, 512], F, tag=f"oc{h}",
                                         name=f"oc{s}_{h}")
                        nc.vector.tensor_copy(ocp[:], Os[h][:])
                        osb.append(ocp)
                    for h in range(2):
                        rec = small.tile([1, 512], F, tag="rec",
                                         name=f"rec{s}_{h}")
                        nc.vector.reciprocal(rec[:], osb[h][64:65, :])
                        recs.append(rec)
                pending_norm = make_norm(p, qc, osb, recs, s, final=final)
        pending_norm()

    nc.compile()
    return nc


def _get_program():
    global _PROGRAM
    if _PROGRAM is None:
        _PROGRAM = _build_program()
    return _PROGRAM


def _prep_core_inputs(inputs, Wq, bq, Wk, bk, Wv, bv, core):
    b, g = divmod(core, 2)
    hs = slice(g * FPC, (g + 1) * FPC)
    xt = np.ascontiguousarray(inputs[b].T).astype(BF16)
    wqt = np.ascontiguousarray(Wq[hs, :].T).astype(BF16)
    wkt = np.ascontiguousarray(Wk[hs, :].T).astype(BF16)
    wvt = np.zeros((D, VW), dtype=BF16)
    for l in range(HPC):
        gh = g * HPC + l
        wvt[:, l * 65:l * 65 + 64] = Wv[gh * 64:(gh + 1) * 64, :].T.astype(BF16)
    bqk = np.stack([np.asarray(bq[hs], dtype=np.float32),
                    np.asarray(bk[hs], dtype=np.float32),
                    np.asarray(bv[hs], dtype=np.float32)], axis=1)
    return {
        "xt": xt,
        "wqt": wqt,
        "wkt": wkt,
        "wvt": wvt,
        "bqk": np.ascontiguousarray(bqk),
    }


def kernel(inputs, Wq, bq, Wk, bk, Wv, bv, _trace=False):
    from concourse.bass_utils import run_bass_kernel_spmd

    inputs = np.asarray(inputs, dtype=np.float32)
    Wq, Wk, Wv = (np.asarray(w, dtype=np.float32) for w in (Wq, Wk, Wv))
    bq, bk, bv = (np.asarray(b, dtype=np.float32) for b in (bq, bk, bv))
    in_maps = [
        _prep_core_inputs(inputs, Wq, bq, Wk, bk, Wv, bv, c) for c in range(N_CORES)
    ]
    nc = _get_program()
    res = run_bass_kernel_spmd(nc, in_maps, list(range(N_CORES)), trace=_trace)
    full = np.empty((B, S, D), dtype=np.float32)
    for c in range(N_CORES):
        b, g = divmod(c, 2)
        full[b, :, g * FPC:(g + 1) * FPC] = res.results[c]["out"].T
    if _trace:
        return full, res
    return full

